# revision 38
# baseline (speedup 1.0000x reference)
"""Trainium2 Bass kernel for nn_BRGEHHNet (gnn_message_passing).

Contract: kernel(**inputs) takes FULL unsharded inputs (as produced by
setup_inputs) and returns the FULL (25, 2048) float32 output.

Strategy: data-parallel over the batch dim across 8 NeuronCores.
Each core handles a 256-column batch shard. BatchNorm statistics are
over the full batch, so every core loads the full transposed states
(bf16) and computes the stats locally (a cross-core allreduce has a
~20us latency floor -- worse than the extra load).

Performance structure (from trace analysis):
  - Matmuls that accumulate back-to-back into the same PSUM bank
    serialize on the array drain (~210ns vs ~109ns cadence), so the
    M1 k-chains of two e-tiles are interleaved into ping-pong PSUM
    banks.
  - BatchNorm stats are the serial head: split across the vector
    engine (bn_stats, tiles 0/2/4/6) and the scalar engine
    (Square/Copy with accum_out, tiles 1/3/5).
  - states stream on the sync (HWDGE) ring, weights on the gpsimd
    (SWDGE) ring, so descriptor generation overlaps.
  - All weights are pre-tiled bf16 host-side; each DMA is a plain 2D
    transfer (1 descriptor per partition).

Math notes:
  - The ANOVA attention (anova + adjacency scatter -> all_att) depends
    only on weight inputs, so it is folded host-side into w1:
    w1eff[e, a*12+k] = all_att[e, a] * w1[a, e, k].
  - w2/w3 per-agent critics become block-diagonal matmuls.
  - Biases ride the scalar-engine activation (out = f(in*scale+bias)).
  - The action gather is a one-hot mask multiply on the vector engine.
"""

import os
import numpy as np
import ml_dtypes

import concourse.bacc as bacc
import concourse.mybir as mybir
import concourse.tile as tile
from concourse import bass_utils

N_CORES = 8
A = 25          # agents
B = 2048        # batch
S = 32          # state dim
F = A * S       # 800 features (contraction of M1)
KT = 7          # f tiles: 6 x 128 + 1 x 32
FR = 32         # ragged tile rows
E = 3200        # EHH_HID (= 25 * 128)
E_MT = E // 128  # 25 output tiles of M1
R = A * 12      # 300 critic hidden rows
NA = 4
BSH = B // N_CORES  # 256 per-core batch shard

# R split in 100s: 100 rounds up to a full 128 PE tile, so no matmul
# drops to a 64/32 tiling mode (mode switches drain the array)
R_SPLIT = [(0, 100), (100, 200), (200, 300)]
# e-tile chunking of the ehh_w / w1eff streams (pipelined DMA)
W_CH = [(0, 2), (2, 7), (7, 12), (12, 17), (17, 22), (22, 25)]
W1_CH = [(0, 7), (7, 16), (16, 25)]
W1_AFTER = {1: 0, 2: 1, 4: 2}   # after wt chunk g, issue w1 chunk

DT = mybir.dt
F32 = DT.float32
BF16 = DT.bfloat16
I32 = DT.int32

TRACE = os.environ.get("BASS_KERNEL_TRACE", "0") == "1"
LAST_EXEC_NS = None

_CACHE = {}


def _build_program():
    nc = bacc.Bacc("TRN2", target_bir_lowering=False, debug=False,
                   num_devices=N_CORES)

    sT_d = nc.dram_tensor("sTt", [128, 6 * B], BF16, kind="ExternalInput")
    sT6_d = nc.dram_tensor("sT6", [FR, B], BF16, kind="ExternalInput")
    wt_d, wt6_d, w1_d = {}, {}, {}
    for g, (m0, m1) in enumerate(W_CH):
        wt_d[g] = nc.dram_tensor(f"wt{g}", [128, 6 * (m1 - m0) * 128], BF16,
                                 kind="ExternalInput")
    wt6all_d = nc.dram_tensor("wt6all", [128, E], BF16, kind="ExternalInput")
    for g, (m0, m1) in enumerate(W1_CH):
        w1_d[g] = nc.dram_tensor(f"w1t{g}", [128, (m1 - m0) * R], BF16,
                                 kind="ExternalInput")
    bd_d = nc.dram_tensor("bdpack", [128, 3 * R + 3 * 128], BF16,
                          kind="ExternalInput")
    bias_d = nc.dram_tensor("biascol", [128, 7], F32, kind="ExternalInput")
    act_d = nc.dram_tensor("act", [A, BSH], I32, kind="ExternalInput")
    out_d = nc.dram_tensor("out", [A, BSH], F32, kind="ExternalOutput")

    with tile.TileContext(nc) as tc:
        with (
            tc.tile_pool(name="const", bufs=1) as cpool,
            tc.tile_pool(name="st", bufs=4) as st_pool,
            tc.tile_pool(name="wf", bufs=len(W_CH)) as wf_pool,
            tc.tile_pool(name="w1", bufs=len(W1_CH)) as w1_pool,
            tc.tile_pool(name="emb", bufs=6) as emb_pool,
            tc.tile_pool(name="hh", bufs=6) as h_pool,
            tc.tile_pool(name="ps", bufs=4, space="PSUM") as ps_pool,
            tc.tile_pool(name="psh1", bufs=3, space="PSUM") as psh1_pool,
        ):
            # ---- states stream first on the gpsimd ring (FIFO priority:
            # the stats head owns the full HBM bandwidth) ----
            stile = cpool.tile([128, 6 * B], BF16, tag="stile")
            st6 = cpool.tile([FR, B], BF16, tag="st6")
            xn = cpool.tile([128, 6 * BSH], BF16, tag="xn")
            xn6 = cpool.tile([128, BSH], BF16, tag="xn6")
            for p0 in (32, 64, 96):
                nc.vector.memset(xn6[p0:p0 + 32, :], 0.0)
            def s_dma(k):
                nc.gpsimd.dma_start(stile[:, k * B:(k + 1) * B],
                                    sT_d.ap()[:, k * B:(k + 1) * B])

            wfc, w1c = {}, {}
            wt6_t = cpool.tile([128, E], BF16, tag="wt6")

            def wt_dma(g):
                m0, m1 = W_CH[g]
                t = wf_pool.tile([128, 6 * (m1 - m0) * 128], BF16, tag="wf",
                                 name=f"wfc_{g}")
                nc.gpsimd.dma_start(t[:], wt_d[g].ap())
                wfc[g] = t

            def w1_dma(g1):
                n1 = W1_CH[g1][1] - W1_CH[g1][0]
                t1 = w1_pool.tile([128, n1 * R], BF16, tag="w1",
                                  name=f"w1c_{g1}")
                nc.gpsimd.dma_start(t1[:], w1_d[g1].ap())
                w1c[g1] = t1

            # ring order: states keep priority, but the first-needed weight
            # tiles (ragged k6 + e-tiles 0-1) slip into the stats slack so
            # the early M1 chains can start while stats are still running
            nc.gpsimd.dma_start(st6[:], sT6_d.ap())
            for _k in range(6):
                s_dma(_k)
            wt_dma(0)
            nc.gpsimd.dma_start(wt6_t[:], wt6all_d.ap())
            wt_dma(1)
            w1_dma(0)
            wt_dma(2)
            wt_dma(3)
            w1_dma(1)
            wt_dma(4)
            wt_dma(5)
            w1_dma(2)
            act_i = cpool.tile([A, BSH], I32, tag="acti")
            nc.sync.dma_start(act_i[:], act_d.ap())
            bd_t = cpool.tile([128, 3 * R + 3 * 128], BF16, tag="bd")
            bias_t = cpool.tile([128, 7], F32, tag="bias")
            nc.gpsimd.dma_start(bd_t[:], bd_d.ap())
            nc.gpsimd.dma_start(bias_t[:], bias_d.ap())

            # ---- batch-norm stats: DVE bn_stats on tiles 6/0/1/3/5, the
            # scalar engine computes sum & sum-of-squares for tiles 2/4
            # via Square/Copy passes with accum_out, epilogue on DVE ----
            ACT_TILES = (2, 4)

            def tile_src(k):
                rows = FR if k == 6 else 128
                xt = st6[:] if k == 6 else stile[:, k * B:(k + 1) * B]
                xno = xn6[:] if k == 6 else xn[:, k * BSH:(k + 1) * BSH]
                return rows, xt, xno

            acc_sq, acc_s = {}, {}
            for k in ACT_TILES:
                rows, xt, _ = tile_src(k)
                acc_sq[k] = st_pool.tile([128, 1], F32, tag="acq",
                                         name=f"accq_{k}")
                acc_s[k] = st_pool.tile([128, 1], F32, tag="acs",
                                        name=f"accs_{k}")
                dq = st_pool.tile([128, B], BF16, tag="dump")
                nc.scalar.activation(dq[0:rows, :], xt[0:rows, :],
                                     mybir.ActivationFunctionType.Square,
                                     accum_out=acc_sq[k][0:rows, :])
                dc = st_pool.tile([128, B], BF16, tag="dump")
                nc.scalar.activation(dc[0:rows, :], xt[0:rows, :],
                                     mybir.ActivationFunctionType.Copy,
                                     accum_out=acc_s[k][0:rows, :])

            def finish(ssum, rows, xt, xno):
                # ssum cols: 0=mean 1=var+eps 2=sigma 3=1/sigma
                nc.scalar.activation(
                    ssum[0:rows, 2:3], ssum[0:rows, 1:2],
                    mybir.ActivationFunctionType.Sqrt)
                nc.vector.reciprocal(ssum[0:rows, 3:4], ssum[0:rows, 2:3])
                nc.vector.tensor_scalar(
                    xno[0:rows, :], xt[0:rows, 0:BSH],
                    ssum[0:rows, 0:1], ssum[0:rows, 3:4],
                    op0=mybir.AluOpType.subtract, op1=mybir.AluOpType.mult)

            inv_b = 1.0 / B

            def dve_bn(k):
                rows, xt, xno = tile_src(k)
                ssum = st_pool.tile([128, 4], F32, tag="st")
                bnst = st_pool.tile([128, 24], F32, tag="bnst")
                for g4 in range(4):
                    nc.vector.bn_stats(
                        bnst[0:rows, 6 * g4:6 * g4 + 6],
                        xt[0:rows, 512 * g4:512 * (g4 + 1)])
                nc.vector.bn_aggr(ssum[0:rows, 0:2], bnst[0:rows, :])
                nc.vector.tensor_scalar(
                    ssum[0:rows, 1:2], ssum[0:rows, 1:2], 1e-5, None,
                    op0=mybir.AluOpType.add)
                finish(ssum, rows, xt, xno)

            def act_epi(k):
                rows, xt, xno = tile_src(k)
                ssum = st_pool.tile([128, 4], F32, tag="st")
                nc.vector.tensor_scalar(
                    ssum[0:rows, 0:1], acc_s[k][0:rows, :], inv_b, None,
                    op0=mybir.AluOpType.mult)
                # var+eps = sumsq/B + eps - mean^2
                nc.vector.tensor_scalar(
                    ssum[0:rows, 1:2], acc_sq[k][0:rows, :], inv_b, 1e-5,
                    op0=mybir.AluOpType.mult, op1=mybir.AluOpType.add)
                nc.vector.tensor_tensor(
                    out=ssum[0:rows, 2:3], in0=ssum[0:rows, 0:1],
                    in1=ssum[0:rows, 0:1], op=mybir.AluOpType.mult)
                nc.vector.tensor_tensor(
                    out=ssum[0:rows, 1:2], in0=ssum[0:rows, 1:2],
                    in1=ssum[0:rows, 2:3], op=mybir.AluOpType.subtract)
                finish(ssum, rows, xt, xno)

            # epilogues for the ACT tiles interleaved between the DVE
            # chains, right where their accumulations become ready
            dve_bn(6)
            dve_bn(0)
            dve_bn(1)
            dve_bn(3)
            dve_bn(5)
            act_epi(2)
            act_epi(4)

            # masks for the action gather (on gpsimd/Pool so the vector
            # engine stays dedicated to the batch-norm stats)
            act_f = cpool.tile([A, BSH], F32, tag="actf")
            nc.gpsimd.tensor_copy(act_f[:], act_i[:])
            masks = []
            for c4 in range(NA):
                mask = cpool.tile([A, BSH], BF16, tag=f"mask_{c4}",
                                  name=f"mask_{c4}")
                nc.gpsimd.tensor_scalar(
                    mask[:], act_f[:], float(c4), None,
                    op0=mybir.AluOpType.is_equal)
                masks.append(mask)

            def wt_lhsT(mt, k):
                if k == 6:
                    return wt6_t[:, mt * 128:(mt + 1) * 128]
                for g, (m0, m1) in enumerate(W_CH):
                    if m0 <= mt < m1:
                        nm = m1 - m0
                        off = (k * nm + (mt - m0)) * 128
                        return wfc[g][:, off:off + 128]
                raise AssertionError

            def w1_lhsT(mt, c0, c1):
                for g1, (m0, m1) in enumerate(W1_CH):
                    if m0 <= mt < m1:
                        off = (mt - m0) * R
                        return w1c[g1][:, off + c0:off + c1]
                raise AssertionError

            # ---- main loop: M1 e-tile PAIRS (ping-pong across FULL PSUM
            # banks -- a 1KB tile would share a 2KB bank with its pair and
            # reintroduce the accumulation-drain serialization) with M2
            # pipelined one pair behind. The k order matches the stats
            # completion order (ragged tile first, tile 5 last) and the
            # first two pairs pre-run their early k-chains to overlap the
            # stats window. ----
            K_ORD = [6, 0, 1, 3, 5, 2, 4]   # stats completion order
            h1ps = [psh1_pool.tile([128, 512], F32, tag="h1ps",
                                   name=f"h1ps_{j}") for j in range(3)]
            embs = []

            def emit_m2(mt):
                for j, (c0, c1) in enumerate(R_SPLIT):
                    nc.tensor.matmul(h1ps[j][0:c1 - c0, 0:BSH],
                                     w1_lhsT(mt, c0, c1), embs[mt][:],
                                     start=(mt == 0), stop=(mt == E_MT - 1))

            def m1_mm(pss, mt, k):
                rhs = xn6[:] if k == 6 else xn[:, k * BSH:(k + 1) * BSH]
                nc.tensor.matmul(pss[mt][:, 0:BSH],
                                 wt_lhsT(mt, k), rhs,
                                 start=(k == K_ORD[0]), stop=(k == K_ORD[-1]))

            prev = []
            pairs = [(2 * i, 2 * i + 1) for i in range(12)] + [(24,)]
            # pairs 0-1: run k 6/0/1/2/3/4 while the stats tail (tile 5)
            # is still computing; k=5 joins at the top of the main loop
            pss01 = {}
            for mts in pairs[0:2]:
                for mt in mts:
                    pss01[mt] = ps_pool.tile([128, 512], F32, tag="ps",
                                             name=f"psm_{mt}")
                for k in K_ORD[:-1]:
                    for mt in mts:
                        m1_mm(pss01, mt, k)
            for mts in pairs[0:2]:
                for mt in mts:
                    m1_mm(pss01, mt, K_ORD[-1])
                for mt in mts:
                    emb = emb_pool.tile([128, BSH], BF16, tag="emb")
                    nc.scalar.activation(emb[:], pss01[mt][:, 0:BSH],
                                         mybir.ActivationFunctionType.Lrelu,
                                         alpha=0.01)
                    embs.append(emb)
                for mt in prev:
                    emit_m2(mt)
                prev = mts
            for mts in pairs[2:]:
                pss = {}
                for mt in mts:
                    pss[mt] = ps_pool.tile([128, 512], F32, tag="ps",
                                           name=f"psm_{mt}")
                for k in K_ORD:
                    for mt in mts:
                        m1_mm(pss, mt, k)
                for mt in mts:
                    emb = emb_pool.tile([128, BSH], BF16, tag="emb")
                    nc.scalar.activation(emb[:], pss[mt][:, 0:BSH],
                                         mybir.ActivationFunctionType.Lrelu,
                                         alpha=0.01)
                    embs.append(emb)
                for mt in prev:
                    emit_m2(mt)
                prev = mts
            for mt in prev:
                emit_m2(mt)

            # finish M2: fused bias + leaky on the scalar engine
            h1 = []
            for j, (c0, c1) in enumerate(R_SPLIT):
                w = c1 - c0
                t = h_pool.tile([128, BSH], BF16, tag=f"h1_{j}",
                                name=f"h1_{j}")
                nc.scalar.activation(t[0:w, :], h1ps[j][0:w, 0:BSH],
                                     mybir.ActivationFunctionType.Lrelu,
                                     bias=bias_t[0:w, j:j + 1], alpha=0.01)
                h1.append(t)

            # M3: h2 = leaky(BD2^T @ h1 + b2), j-interleaved for bank spacing
            ps3 = [ps_pool.tile([128, 512], F32, tag="ps", name=f"ps3_{j}")
                   for j in range(3)]
            for k3, (k0, k1) in enumerate(R_SPLIT):
                for j, (c0, c1) in enumerate(R_SPLIT):
                    nc.tensor.matmul(
                        ps3[j][0:c1 - c0, 0:BSH],
                        bd_t[0:k1 - k0, k3 * R + c0:k3 * R + c1],
                        h1[k3][0:k1 - k0, :],
                        start=(k3 == 0), stop=(k3 == 2))
            h2 = []
            for j, (c0, c1) in enumerate(R_SPLIT):
                w = c1 - c0
                t = h_pool.tile([128, BSH], BF16, tag=f"h2_{j}",
                                name=f"h2_{j}")
                nc.scalar.activation(t[0:w, :], ps3[j][0:w, 0:BSH],
                                     mybir.ActivationFunctionType.Lrelu,
                                     bias=bias_t[0:w, 3 + j:4 + j], alpha=0.01)
                h2.append(t)

            # M4: all_q^T (rows = c*32+a) = BD3^T @ h2 + b3
            ps_q = ps_pool.tile([128, 512], F32, tag="ps", name="psq")
            for k4, (k0, k1) in enumerate(R_SPLIT):
                nc.tensor.matmul(
                    ps_q[:, 0:BSH],
                    bd_t[0:k1 - k0, 3 * R + k4 * 128:3 * R + (k4 + 1) * 128],
                    h2[k4][0:k1 - k0, :],
                    start=(k4 == 0), stop=(k4 == 2))

            # b3 bias in place on PSUM (partition-aligned), then gather
            nc.vector.tensor_scalar(ps_q[:, 0:BSH], ps_q[:, 0:BSH],
                                    bias_t[:, 6:7], None,
                                    op0=mybir.AluOpType.add)
            qs = []
            for c4 in range(NA):
                qc = cpool.tile([A, BSH], BF16, tag=f"qc_{c4}",
                                name=f"qc_{c4}")
                nc.vector.tensor_tensor(
                    out=qc[:], in0=ps_q[c4 * 32:c4 * 32 + A, 0:BSH],
                    in1=masks[c4][:], op=mybir.AluOpType.mult)
                qs.append(qc)
            nc.vector.tensor_tensor(out=qs[0][:], in0=qs[0][:], in1=qs[1][:],
                                    op=mybir.AluOpType.add)
            nc.vector.tensor_tensor(out=qs[2][:], in0=qs[2][:], in1=qs[3][:],
                                    op=mybir.AluOpType.add)
            qf = cpool.tile([A, BSH], F32, tag="qf")
            nc.vector.tensor_tensor(out=qf[:], in0=qs[0][:], in1=qs[2][:],
                                    op=mybir.AluOpType.add)
            nc.sync.dma_start(out_d.ap(), qf[:])

    nc.compile()
    return nc


def _host_prep(inputs):
    bf16 = ml_dtypes.bfloat16
    states = np.asarray(inputs["states"], dtype=np.float32)
    ehh_w = np.asarray(inputs["ehh_w"], dtype=np.float32)
    anova = np.asarray(inputs["anova"], dtype=np.float32)
    w1 = np.asarray(inputs["w1"], dtype=np.float32)
    b1 = np.asarray(inputs["b1"], dtype=np.float32)
    w2 = np.asarray(inputs["w2"], dtype=np.float32)
    b2 = np.asarray(inputs["b2"], dtype=np.float32)
    w3 = np.asarray(inputs["w3"], dtype=np.float32)
    b3 = np.asarray(inputs["b3"], dtype=np.float32)
    actions = np.asarray(inputs["actions"], dtype=np.int32)
    adj = np.asarray(inputs["adj"], dtype=np.int64)

    sT = states.transpose(0, 2, 1).reshape(F, B)

    # adjacency scatter -> all_att (weight-only; fold into w1)
    src = np.full(E, -1, dtype=np.int64)
    for e in range(adj.shape[0]):
        src[adj[e, 1]] = adj[e, 0]
    for e in range(adj.shape[0]):
        src[adj[e, 3]] = adj[e, 0]
    neighbor = np.zeros((E, A), dtype=np.float32)
    hit = np.nonzero(src >= 0)[0]
    neighbor[hit] = anova[E + src[hit], :]
    all_att = anova[:E, :] + neighbor                     # (E, A)
    # w1eff[e, a*12+k] = all_att[e, a] * w1[a, e, k]
    w1eff = (w1.transpose(1, 0, 2) * all_att[:, :, None]).reshape(E, R)

    # pretiled bf16 payloads
    wt4 = ehh_w[:768].reshape(6, 128, E_MT, 128)          # [k, p, m, j]
    wt_maps = {}
    for g, (m0, m1) in enumerate(W_CH):
        c = wt4[:, :, m0:m1, :].transpose(1, 0, 2, 3)     # [p, k, m, j]
        wt_maps[f"wt{g}"] = np.ascontiguousarray(
            c.reshape(128, -1)).astype(bf16)
    wt6p = np.zeros((128, E), dtype=np.float32)
    wt6p[:FR] = ehh_w[768:]
    wt_maps["wt6all"] = wt6p.astype(bf16)
    w14 = w1eff.reshape(E_MT, 128, R)                     # [m, p, r]
    for g, (m0, m1) in enumerate(W1_CH):
        c = w14[m0:m1].transpose(1, 0, 2)                 # [p, m, r]
        wt_maps[f"w1t{g}"] = np.ascontiguousarray(
            c.reshape(128, -1)).astype(bf16)

    bd = np.zeros((128, 3 * R + 3 * 128), dtype=np.float32)
    biascol = np.zeros((128, 7), dtype=np.float32)
    for a in range(A):
        r0 = 12 * a
        for k3, (k0, k1) in enumerate(R_SPLIT):
            lo = max(r0, k0)
            hi = min(r0 + 12, k1)
            if lo >= hi:
                continue
            bd[lo - k0:hi - k0, k3 * R + r0:k3 * R + r0 + 12] = \
                w2[a][lo - r0:hi - r0, :]
            for c in range(NA):
                bd[lo - k0:hi - k0, 3 * R + k3 * 128 + c * 32 + a] = \
                    w3[a, lo - r0:hi - r0, c]
    b1r = b1.reshape(R)
    b2r = b2.reshape(R)
    for j, (c0, c1) in enumerate(R_SPLIT):
        biascol[0:c1 - c0, j] = b1r[c0:c1]
        biascol[0:c1 - c0, 3 + j] = b2r[c0:c1]
    for c in range(NA):
        biascol[c * 32:c * 32 + A, 6] = b3[:, c]

    common = {
        "bdpack": bd.astype(bf16),
        "biascol": biascol,
    }
    common.update(wt_maps)
    in_maps = []
    for c in range(N_CORES):
        m = dict(common)
        rolled = np.roll(sT, -BSH * c, axis=1)            # own shard first
        m["sTt"] = np.ascontiguousarray(
            rolled[:768].reshape(6, 128, B).transpose(1, 0, 2)
            .reshape(128, 6 * B)).astype(bf16)
        m["sT6"] = np.ascontiguousarray(rolled[768:]).astype(bf16)
        m["act"] = np.ascontiguousarray(actions[:, BSH * c:BSH * (c + 1)])
        in_maps.append(m)
    return in_maps


def kernel(**inputs):
    global LAST_EXEC_NS
    if "nc" not in _CACHE:
        _CACHE["nc"] = _build_program()
    nc = _CACHE["nc"]
    in_maps = _host_prep(inputs)
    kwargs = {}
    if TRACE:
        kwargs["trace"] = True
    res = bass_utils.run_bass_kernel_spmd(
        nc, in_maps, core_ids=list(range(N_CORES)), **kwargs)
    LAST_EXEC_NS = res.exec_time_ns
    q = np.empty((A, B), dtype=np.float32)
    for c in range(N_CORES):
        q[:, BSH * c:BSH * (c + 1)] = res.results[c]["out"]
    return q


# revision 41
# speedup vs baseline: 1.0193x; 1.0193x over previous
"""Trainium2 Bass kernel for nn_BRGEHHNet (gnn_message_passing).

Contract: kernel(**inputs) takes FULL unsharded inputs (as produced by
setup_inputs) and returns the FULL (25, 2048) float32 output.

Strategy: data-parallel over the batch dim across 8 NeuronCores.
Each core handles a 256-column batch shard. BatchNorm statistics are
over the full batch, so every core loads the full transposed states
(bf16) and computes the stats locally (a cross-core allreduce has a
~20us latency floor -- worse than the extra load).

Performance structure (from trace analysis):
  - Matmuls that accumulate back-to-back into the same PSUM bank
    serialize on the array drain (~210ns vs ~109ns cadence), so the
    M1 k-chains of two e-tiles are interleaved into ping-pong PSUM
    banks.
  - BatchNorm stats are the serial head: split across the vector
    engine (bn_stats, tiles 0/2/4/6) and the scalar engine
    (Square/Copy with accum_out, tiles 1/3/5).
  - states stream on the sync (HWDGE) ring, weights on the gpsimd
    (SWDGE) ring, so descriptor generation overlaps.
  - All weights are pre-tiled bf16 host-side; each DMA is a plain 2D
    transfer (1 descriptor per partition).

Math notes:
  - The ANOVA attention (anova + adjacency scatter -> all_att) depends
    only on weight inputs, so it is folded host-side into w1:
    w1eff[e, a*12+k] = all_att[e, a] * w1[a, e, k].
  - w2/w3 per-agent critics become block-diagonal matmuls.
  - Biases ride the scalar-engine activation (out = f(in*scale+bias)).
  - The action gather is a one-hot mask multiply on the vector engine.
"""

import os
import numpy as np
import ml_dtypes

import concourse.bacc as bacc
import concourse.mybir as mybir
import concourse.tile as tile
from concourse import bass_utils

N_CORES = 8
A = 25          # agents
B = 2048        # batch
S = 32          # state dim
F = A * S       # 800 features (contraction of M1)
KT = 7          # f tiles: 6 x 128 + 1 x 32
FR = 32         # ragged tile rows
E = 3200        # EHH_HID (= 25 * 128)
E_MT = E // 128  # 25 output tiles of M1
R = A * 12      # 300 critic hidden rows
NA = 4
BSH = B // N_CORES  # 256 per-core batch shard

# R split in 100s: 100 rounds up to a full 128 PE tile, so no matmul
# drops to a 64/32 tiling mode (mode switches drain the array)
R_SPLIT = [(0, 100), (100, 200), (200, 300)]
# e-tile chunking of the ehh_w / w1eff streams (pipelined DMA)
W_CH = [(0, 2), (2, 7), (7, 12), (12, 17), (17, 22), (22, 25)]
W1_CH = [(0, 7), (7, 16), (16, 25)]
W1_AFTER = {1: 0, 2: 1, 4: 2}   # after wt chunk g, issue w1 chunk

DT = mybir.dt
F32 = DT.float32
BF16 = DT.bfloat16
I32 = DT.int32

TRACE = os.environ.get("BASS_KERNEL_TRACE", "0") == "1"
LAST_EXEC_NS = None

_CACHE = {}


def _build_program():
    nc = bacc.Bacc("TRN2", target_bir_lowering=False, debug=False,
                   num_devices=N_CORES)

    sT_d = nc.dram_tensor("sTt", [128, 6 * B], BF16, kind="ExternalInput")
    sT6_d = nc.dram_tensor("sT6", [FR, B], BF16, kind="ExternalInput")
    wt_d, wt6_d, w1_d = {}, {}, {}
    for g, (m0, m1) in enumerate(W_CH):
        wt_d[g] = nc.dram_tensor(f"wt{g}", [128, 6 * (m1 - m0) * 128], BF16,
                                 kind="ExternalInput")
    wt6all_d = nc.dram_tensor("wt6all", [128, E], BF16, kind="ExternalInput")
    for g, (m0, m1) in enumerate(W1_CH):
        w1_d[g] = nc.dram_tensor(f"w1t{g}", [128, (m1 - m0) * R], BF16,
                                 kind="ExternalInput")
    bd_d = nc.dram_tensor("bdpack", [128, 3 * R + 3 * 128], BF16,
                          kind="ExternalInput")
    bias_d = nc.dram_tensor("biascol", [128, 7], F32, kind="ExternalInput")
    act_d = nc.dram_tensor("act", [A, BSH], I32, kind="ExternalInput")
    out_d = nc.dram_tensor("out", [A, BSH], F32, kind="ExternalOutput")

    with tile.TileContext(nc) as tc:
        with (
            tc.tile_pool(name="const", bufs=1) as cpool,
            tc.tile_pool(name="st", bufs=4) as st_pool,
            tc.tile_pool(name="wf", bufs=len(W_CH)) as wf_pool,
            tc.tile_pool(name="w1", bufs=len(W1_CH)) as w1_pool,
            tc.tile_pool(name="emb", bufs=6) as emb_pool,
            tc.tile_pool(name="hh", bufs=6) as h_pool,
            tc.tile_pool(name="ps", bufs=4, space="PSUM") as ps_pool,
            tc.tile_pool(name="psh1", bufs=3, space="PSUM") as psh1_pool,
        ):
            # ---- states stream first on the gpsimd ring (FIFO priority:
            # the stats head owns the full HBM bandwidth) ----
            stile = cpool.tile([128, 6 * B], BF16, tag="stile")
            st6 = cpool.tile([FR, B], BF16, tag="st6")
            xn = cpool.tile([128, 6 * BSH], BF16, tag="xn")
            xn6 = cpool.tile([128, BSH], BF16, tag="xn6")
            for p0 in (32, 64, 96):
                nc.vector.memset(xn6[p0:p0 + 32, :], 0.0)
            def s_dma(k):
                nc.gpsimd.dma_start(stile[:, k * B:(k + 1) * B],
                                    sT_d.ap()[:, k * B:(k + 1) * B])

            wfc, w1c = {}, {}
            wt6_t = cpool.tile([128, E], BF16, tag="wt6")

            def wt_dma(g):
                m0, m1 = W_CH[g]
                t = wf_pool.tile([128, 6 * (m1 - m0) * 128], BF16, tag="wf",
                                 name=f"wfc_{g}")
                nc.gpsimd.dma_start(t[:], wt_d[g].ap())
                wfc[g] = t

            def w1_dma(g1):
                n1 = W1_CH[g1][1] - W1_CH[g1][0]
                t1 = w1_pool.tile([128, n1 * R], BF16, tag="w1",
                                  name=f"w1c_{g1}")
                nc.gpsimd.dma_start(t1[:], w1_d[g1].ap())
                w1c[g1] = t1

            # ring order: states keep priority, but the first-needed weight
            # tiles (ragged k6 + e-tiles 0-1) slip into the stats slack so
            # the early M1 chains can start while stats are still running
            nc.gpsimd.dma_start(st6[:], sT6_d.ap())
            for _k in range(6):
                s_dma(_k)
            wt_dma(0)
            nc.gpsimd.dma_start(wt6_t[:], wt6all_d.ap())
            wt_dma(1)
            w1_dma(0)
            wt_dma(2)
            wt_dma(3)
            w1_dma(1)
            wt_dma(4)
            wt_dma(5)
            w1_dma(2)
            act_i = cpool.tile([A, BSH], I32, tag="acti")
            nc.sync.dma_start(act_i[:], act_d.ap())
            bd_t = cpool.tile([128, 3 * R + 3 * 128], BF16, tag="bd")
            bias_t = cpool.tile([128, 7], F32, tag="bias")
            nc.gpsimd.dma_start(bd_t[:], bd_d.ap())
            nc.gpsimd.dma_start(bias_t[:], bias_d.ap())

            # ---- batch-norm stats: DVE bn_stats on tiles 6/0/1/3/5, the
            # scalar engine computes sum & sum-of-squares for tiles 2/4
            # via Square/Copy passes with accum_out, epilogue on DVE ----
            ACT_TILES = (2, 4)

            def tile_src(k):
                rows = FR if k == 6 else 128
                xt = st6[:] if k == 6 else stile[:, k * B:(k + 1) * B]
                xno = xn6[:] if k == 6 else xn[:, k * BSH:(k + 1) * BSH]
                return rows, xt, xno

            acc_sq, acc_s = {}, {}
            for k in ACT_TILES:
                rows, xt, _ = tile_src(k)
                acc_sq[k] = st_pool.tile([128, 1], F32, tag="acq",
                                         name=f"accq_{k}")
                acc_s[k] = st_pool.tile([128, 1], F32, tag="acs",
                                        name=f"accs_{k}")
                dq = st_pool.tile([128, B], BF16, tag="dump")
                nc.scalar.activation(dq[0:rows, :], xt[0:rows, :],
                                     mybir.ActivationFunctionType.Square,
                                     accum_out=acc_sq[k][0:rows, :])
                dc = st_pool.tile([128, B], BF16, tag="dump")
                nc.scalar.activation(dc[0:rows, :], xt[0:rows, :],
                                     mybir.ActivationFunctionType.Copy,
                                     accum_out=acc_s[k][0:rows, :])

            def finish(ssum, rows, xt, xno):
                # ssum cols: 0=mean 1=var+eps 2=sigma 3=1/sigma
                nc.scalar.activation(
                    ssum[0:rows, 2:3], ssum[0:rows, 1:2],
                    mybir.ActivationFunctionType.Sqrt)
                nc.vector.reciprocal(ssum[0:rows, 3:4], ssum[0:rows, 2:3])
                nc.vector.tensor_scalar(
                    xno[0:rows, :], xt[0:rows, 0:BSH],
                    ssum[0:rows, 0:1], ssum[0:rows, 3:4],
                    op0=mybir.AluOpType.subtract, op1=mybir.AluOpType.mult)

            inv_b = 1.0 / B

            def dve_bn(k):
                rows, xt, xno = tile_src(k)
                ssum = st_pool.tile([128, 4], F32, tag="st")
                bnst = st_pool.tile([128, 24], F32, tag="bnst")
                for g4 in range(4):
                    nc.vector.bn_stats(
                        bnst[0:rows, 6 * g4:6 * g4 + 6],
                        xt[0:rows, 512 * g4:512 * (g4 + 1)])
                nc.vector.bn_aggr(ssum[0:rows, 0:2], bnst[0:rows, :])
                nc.vector.tensor_scalar(
                    ssum[0:rows, 1:2], ssum[0:rows, 1:2], 1e-5, None,
                    op0=mybir.AluOpType.add)
                finish(ssum, rows, xt, xno)

            def act_epi(k):
                rows, xt, xno = tile_src(k)
                ssum = st_pool.tile([128, 4], F32, tag="st")
                nc.vector.tensor_scalar(
                    ssum[0:rows, 0:1], acc_s[k][0:rows, :], inv_b, None,
                    op0=mybir.AluOpType.mult)
                # var+eps = sumsq/B + eps - mean^2
                nc.vector.tensor_scalar(
                    ssum[0:rows, 1:2], acc_sq[k][0:rows, :], inv_b, 1e-5,
                    op0=mybir.AluOpType.mult, op1=mybir.AluOpType.add)
                nc.vector.tensor_tensor(
                    out=ssum[0:rows, 2:3], in0=ssum[0:rows, 0:1],
                    in1=ssum[0:rows, 0:1], op=mybir.AluOpType.mult)
                nc.vector.tensor_tensor(
                    out=ssum[0:rows, 1:2], in0=ssum[0:rows, 1:2],
                    in1=ssum[0:rows, 2:3], op=mybir.AluOpType.subtract)
                finish(ssum, rows, xt, xno)

            # epilogues for the ACT tiles interleaved between the DVE
            # chains, right where their accumulations become ready
            dve_bn(6)
            dve_bn(0)
            dve_bn(1)
            dve_bn(3)
            dve_bn(5)
            act_epi(2)
            act_epi(4)

            # masks for the action gather (on gpsimd/Pool so the vector
            # engine stays dedicated to the batch-norm stats)
            act_f = cpool.tile([A, BSH], F32, tag="actf")
            nc.gpsimd.tensor_copy(act_f[:], act_i[:])
            masks = []
            for c4 in range(NA):
                mask = cpool.tile([A, BSH], BF16, tag=f"mask_{c4}",
                                  name=f"mask_{c4}")
                nc.gpsimd.tensor_scalar(
                    mask[:], act_f[:], float(c4), None,
                    op0=mybir.AluOpType.is_equal)
                masks.append(mask)

            def wt_lhsT(mt, k):
                if k == 6:
                    return wt6_t[:, mt * 128:(mt + 1) * 128]
                for g, (m0, m1) in enumerate(W_CH):
                    if m0 <= mt < m1:
                        nm = m1 - m0
                        off = (k * nm + (mt - m0)) * 128
                        return wfc[g][:, off:off + 128]
                raise AssertionError

            def w1_lhsT(mt, c0, c1):
                for g1, (m0, m1) in enumerate(W1_CH):
                    if m0 <= mt < m1:
                        off = (mt - m0) * R
                        return w1c[g1][:, off + c0:off + c1]
                raise AssertionError

            # ---- main loop: M1 e-tile PAIRS (ping-pong across FULL PSUM
            # banks -- a 1KB tile would share a 2KB bank with its pair and
            # reintroduce the accumulation-drain serialization) with M2
            # pipelined one pair behind. The k order matches the stats
            # completion order (ragged tile first, tile 5 last) and the
            # first two pairs pre-run their early k-chains to overlap the
            # stats window. ----
            K_ORD = [6, 0, 1, 3, 5, 2, 4]   # stats completion order
            h1ps = [psh1_pool.tile([128, 512], F32, tag="h1ps",
                                   name=f"h1ps_{j}") for j in range(3)]
            embs = []

            def emit_m2(mt):
                for j, (c0, c1) in enumerate(R_SPLIT):
                    nc.tensor.matmul(h1ps[j][0:c1 - c0, 0:BSH],
                                     w1_lhsT(mt, c0, c1), embs[mt][:],
                                     start=(mt == 0), stop=(mt == E_MT - 1))

            def m1_mm(pss, mt, k):
                rhs = xn6[:] if k == 6 else xn[:, k * BSH:(k + 1) * BSH]
                nc.tensor.matmul(pss[mt][:, 0:BSH],
                                 wt_lhsT(mt, k), rhs,
                                 start=(k == K_ORD[0]), stop=(k == K_ORD[-1]))

            prev = []
            pairs = [(2 * i, 2 * i + 1) for i in range(12)] + [(24,)]
            # pairs 0-1: run k 6/0/1/2/3/4 while the stats tail (tile 5)
            # is still computing; k=5 joins at the top of the main loop
            pss01 = {}
            for mts in pairs[0:2]:
                for mt in mts:
                    pss01[mt] = ps_pool.tile([128, 512], F32, tag="ps",
                                             name=f"psm_{mt}")
                for k in K_ORD[:-1]:
                    for mt in mts:
                        m1_mm(pss01, mt, k)
            for mts in pairs[0:2]:
                for mt in mts:
                    m1_mm(pss01, mt, K_ORD[-1])
                for mt in mts:
                    emb = emb_pool.tile([128, BSH], BF16, tag="emb")
                    nc.scalar.activation(emb[:], pss01[mt][:, 0:BSH],
                                         mybir.ActivationFunctionType.Lrelu,
                                         alpha=0.01)
                    embs.append(emb)
                for mt in prev:
                    emit_m2(mt)
                prev = mts
            for mts in pairs[2:]:
                pss = {}
                for mt in mts:
                    pss[mt] = ps_pool.tile([128, 512], F32, tag="ps",
                                           name=f"psm_{mt}")
                for k in K_ORD:
                    for mt in mts:
                        m1_mm(pss, mt, k)
                for mt in mts:
                    emb = emb_pool.tile([128, BSH], BF16, tag="emb")
                    nc.scalar.activation(emb[:], pss[mt][:, 0:BSH],
                                         mybir.ActivationFunctionType.Lrelu,
                                         alpha=0.01)
                    embs.append(emb)
                for mt in prev:
                    emit_m2(mt)
                prev = mts
            for mt in prev:
                emit_m2(mt)

            # finish M2: fused bias + leaky on the scalar engine
            h1 = []
            for j, (c0, c1) in enumerate(R_SPLIT):
                w = c1 - c0
                t = h_pool.tile([128, BSH], BF16, tag=f"h1_{j}",
                                name=f"h1_{j}")
                nc.scalar.activation(t[0:w, :], h1ps[j][0:w, 0:BSH],
                                     mybir.ActivationFunctionType.Lrelu,
                                     bias=bias_t[0:w, j:j + 1], alpha=0.01)
                h1.append(t)

            # M3: h2 = leaky(BD2^T @ h1 + b2), j-interleaved for bank spacing
            ps3 = [ps_pool.tile([128, 512], F32, tag="ps", name=f"ps3_{j}")
                   for j in range(3)]
            for k3, (k0, k1) in enumerate(R_SPLIT):
                for j, (c0, c1) in enumerate(R_SPLIT):
                    nc.tensor.matmul(
                        ps3[j][0:c1 - c0, 0:BSH],
                        bd_t[0:k1 - k0, k3 * R + c0:k3 * R + c1],
                        h1[k3][0:k1 - k0, :],
                        start=(k3 == 0), stop=(k3 == 2))
            h2 = []
            for j, (c0, c1) in enumerate(R_SPLIT):
                w = c1 - c0
                t = h_pool.tile([128, BSH], BF16, tag=f"h2_{j}",
                                name=f"h2_{j}")
                nc.scalar.activation(t[0:w, :], ps3[j][0:w, 0:BSH],
                                     mybir.ActivationFunctionType.Lrelu,
                                     bias=bias_t[0:w, 3 + j:4 + j], alpha=0.01)
                h2.append(t)

            # M4: all_q^T (rows = c*32+a) = BD3^T @ h2 + b3
            ps_q = ps_pool.tile([128, 512], F32, tag="ps", name="psq")
            for k4, (k0, k1) in enumerate(R_SPLIT):
                nc.tensor.matmul(
                    ps_q[:, 0:BSH],
                    bd_t[0:k1 - k0, 3 * R + k4 * 128:3 * R + (k4 + 1) * 128],
                    h2[k4][0:k1 - k0, :],
                    start=(k4 == 0), stop=(k4 == 2))

            # b3 bias in place on PSUM (partition-aligned), then gather
            nc.vector.tensor_scalar(ps_q[:, 0:BSH], ps_q[:, 0:BSH],
                                    bias_t[:, 6:7], None,
                                    op0=mybir.AluOpType.add)
            qs = []
            for c4 in range(NA):
                qc = cpool.tile([A, BSH], BF16, tag=f"qc_{c4}",
                                name=f"qc_{c4}")
                nc.vector.tensor_tensor(
                    out=qc[:], in0=ps_q[c4 * 32:c4 * 32 + A, 0:BSH],
                    in1=masks[c4][:], op=mybir.AluOpType.mult)
                qs.append(qc)
            nc.vector.tensor_tensor(out=qs[0][:], in0=qs[0][:], in1=qs[1][:],
                                    op=mybir.AluOpType.add)
            nc.vector.tensor_tensor(out=qs[2][:], in0=qs[2][:], in1=qs[3][:],
                                    op=mybir.AluOpType.add)
            qf = cpool.tile([A, BSH], F32, tag="qf")
            nc.vector.tensor_tensor(out=qf[:], in0=qs[0][:], in1=qs[2][:],
                                    op=mybir.AluOpType.add)
            nc.sync.dma_start(out_d.ap(), qf[:])

    nc.compile()
    return nc


def _host_prep(inputs):
    bf16 = ml_dtypes.bfloat16
    states = np.asarray(inputs["states"], dtype=np.float32)
    ehh_w = np.asarray(inputs["ehh_w"], dtype=np.float32)
    anova = np.asarray(inputs["anova"], dtype=np.float32)
    w1 = np.asarray(inputs["w1"], dtype=np.float32)
    b1 = np.asarray(inputs["b1"], dtype=np.float32)
    w2 = np.asarray(inputs["w2"], dtype=np.float32)
    b2 = np.asarray(inputs["b2"], dtype=np.float32)
    w3 = np.asarray(inputs["w3"], dtype=np.float32)
    b3 = np.asarray(inputs["b3"], dtype=np.float32)
    actions = np.asarray(inputs["actions"], dtype=np.int32)
    adj = np.asarray(inputs["adj"], dtype=np.int64)

    sT = states.transpose(0, 2, 1).reshape(F, B)

    # adjacency scatter -> all_att (weight-only; fold into w1)
    src = np.full(E, -1, dtype=np.int64)
    for e in range(adj.shape[0]):
        src[adj[e, 1]] = adj[e, 0]
    for e in range(adj.shape[0]):
        src[adj[e, 3]] = adj[e, 0]
    neighbor = np.zeros((E, A), dtype=np.float32)
    hit = np.nonzero(src >= 0)[0]
    neighbor[hit] = anova[E + src[hit], :]
    all_att = anova[:E, :] + neighbor                     # (E, A)
    # w1eff[e, a*12+k] = all_att[e, a] * w1[a, e, k]
    w1eff = (w1.transpose(1, 0, 2) * all_att[:, :, None]).reshape(E, R)

    # pretiled bf16 payloads
    wt4 = ehh_w[:768].reshape(6, 128, E_MT, 128)          # [k, p, m, j]
    wt_maps = {}
    for g, (m0, m1) in enumerate(W_CH):
        c = wt4[:, :, m0:m1, :].transpose(1, 0, 2, 3)     # [p, k, m, j]
        wt_maps[f"wt{g}"] = np.ascontiguousarray(
            c.reshape(128, -1)).astype(bf16)
    wt6p = np.zeros((128, E), dtype=np.float32)
    wt6p[:FR] = ehh_w[768:]
    wt_maps["wt6all"] = wt6p.astype(bf16)
    w14 = w1eff.reshape(E_MT, 128, R)                     # [m, p, r]
    for g, (m0, m1) in enumerate(W1_CH):
        c = w14[m0:m1].transpose(1, 0, 2)                 # [p, m, r]
        wt_maps[f"w1t{g}"] = np.ascontiguousarray(
            c.reshape(128, -1)).astype(bf16)

    bd = np.zeros((128, 3 * R + 3 * 128), dtype=np.float32)
    biascol = np.zeros((128, 7), dtype=np.float32)
    for a in range(A):
        r0 = 12 * a
        for k3, (k0, k1) in enumerate(R_SPLIT):
            lo = max(r0, k0)
            hi = min(r0 + 12, k1)
            if lo >= hi:
                continue
            bd[lo - k0:hi - k0, k3 * R + r0:k3 * R + r0 + 12] = \
                w2[a][lo - r0:hi - r0, :]
            for c in range(NA):
                bd[lo - k0:hi - k0, 3 * R + k3 * 128 + c * 32 + a] = \
                    w3[a, lo - r0:hi - r0, c]
    b1r = b1.reshape(R)
    b2r = b2.reshape(R)
    for j, (c0, c1) in enumerate(R_SPLIT):
        biascol[0:c1 - c0, j] = b1r[c0:c1]
        biascol[0:c1 - c0, 3 + j] = b2r[c0:c1]
    for c in range(NA):
        biascol[c * 32:c * 32 + A, 6] = b3[:, c]

    common = {
        "bdpack": bd.astype(bf16),
        "biascol": biascol,
    }
    common.update(wt_maps)
    in_maps = []
    for c in range(N_CORES):
        m = dict(common)
        rolled = np.roll(sT, -BSH * c, axis=1)            # own shard first
        m["sTt"] = np.ascontiguousarray(
            rolled[:768].reshape(6, 128, B).transpose(1, 0, 2)
            .reshape(128, 6 * B)).astype(bf16)
        m["sT6"] = np.ascontiguousarray(rolled[768:]).astype(bf16)
        m["act"] = np.ascontiguousarray(actions[:, BSH * c:BSH * (c + 1)])
        in_maps.append(m)
    return in_maps


def kernel(**inputs):
    global LAST_EXEC_NS
    if "nc" not in _CACHE:
        _CACHE["nc"] = _build_program()
    nc = _CACHE["nc"]
    in_maps = _host_prep(inputs)
    kwargs = {}
    if TRACE:
        kwargs["trace"] = True
    res = bass_utils.run_bass_kernel_spmd(
        nc, in_maps, core_ids=list(range(N_CORES)), **kwargs)
    LAST_EXEC_NS = res.exec_time_ns
    q = np.empty((A, B), dtype=np.float32)
    for c in range(N_CORES):
        q[:, BSH * c:BSH * (c + 1)] = res.results[c]["out"]
    return q
